# revision 1
# baseline (speedup 1.0000x reference)
"""ChildSumTreeLSTM on 8 trn2 NeuronCores.

Tree is a reversed complete 4-ary heap (id = N-1-heap, heap j's children are
4j+1..4j+4).  Shard the 64 depth-3 subtrees rooted at heap 21..84 contiguously
across 8 cores (8 subtrees/core).  Each core runs a uniform padded forest
(levels of 512/128/32/8 slots) with zero-padded slots; a leaf is identical to
an internal node whose children rows are zero, so one level-step kernel serves
everything.  One 32KB AllGather moves the 64 subtree roots everywhere, then
every core redundantly computes the 21-node top tree (heap 0..20) and writes
the root h.

All on-device tensors use "T layout": mem dim (512 -> 4 partition tiles of
128) on partitions, node slots on the free dim.  GEMMs are out.T = W.T @ actT
with weight k-tiles as the stationary operand.
"""

import os
import sys

sys.path.insert(0, "/opt/trn_rl_repo")

import numpy as np

import concourse.bass as bass
import concourse.bacc as bacc
import concourse.mybir as mybir
import concourse.tile as tile
from concourse.bass_utils import run_bass_kernel_spmd

F32 = mybir.dt.float32
BF16 = mybir.dt.float16  # GEMM operand dtype (fp16: single-pass PE, 10-bit mantissa)
AF = mybir.ActivationFunctionType
ALU = mybir.AluOpType
AX = mybir.AxisListType

N = 4096
MEM = 512
IN_DIM = 512
NCORES = 8
P = 128
KT = 4  # contraction tiles (512 / 128)

# slot layout in the 704-column per-core node array
NL3, NL2, NL1, NL0 = 512, 128, 32, 8
OFF3, OFF2, OFF1, OFF0 = 0, 512, 640, 672
OFFT2, OFFT1, OFFT0 = 680, 696, 700
NSLOT = 704
NHALF = 352

TOP_ON_DEVICE = os.environ.get("KERNEL_TOP", "device") == "device"

LAST_RESULT = None  # BassKernelResults of the most recent run (for test.py)


def _core_heaps(c):
    t0 = 21 + 8 * c
    heaps = []
    for s in range(8):
        heaps += [64 * (t0 + s) + 21 + a for a in range(64)]  # rel3
    for s in range(8):
        heaps += [16 * (t0 + s) + 5 + a for a in range(16)]  # rel2
    for s in range(8):
        heaps += [4 * (t0 + s) + 1 + a for a in range(4)]  # rel1
    for s in range(8):
        heaps += [t0 + s]  # rel0
    heaps += list(range(5, 21)) + list(range(1, 5)) + [0]  # T2, T1, T0
    heaps += [-1, -1, -1]  # pad to 704
    return np.array(heaps, dtype=np.int64)


def _build_program():
    nc = bacc.Bacc("TRN2", target_bir_lowering=False, debug=False)

    xin_d = nc.dram_tensor("xin", [IN_DIM, NSLOT], BF16, kind="ExternalInput")
    wx_d = nc.dram_tensor("wx", [IN_DIM, 4 * MEM], BF16, kind="ExternalInput")
    ws_d = nc.dram_tensor("ws", [MEM, 3 * MEM], BF16, kind="ExternalInput")
    wf_d = nc.dram_tensor("wf", [MEM, MEM], BF16, kind="ExternalInput")
    bx_d = nc.dram_tensor("bx", [P, 16], F32, kind="ExternalInput")
    bs_d = nc.dram_tensor("bs", [P, 12], F32, kind="ExternalInput")
    bf_d = nc.dram_tensor("bf", [P, 4], F32, kind="ExternalInput")
    cm_d = nc.dram_tensor("cmask", [P, NL3], F32, kind="ExternalInput")
    out_d = nc.dram_tensor("out", [1, MEM], F32, kind="ExternalOutput")
    if not TOP_ON_DEVICE:
        roots_d = nc.dram_tensor("roots", [2 * MEM, NL0], F32, kind="ExternalOutput")
    if TOP_ON_DEVICE:
        contrib_d = nc.dram_tensor("contrib", [2 * MEM, NL0], F32)
        gath_d = nc.dram_tensor("gath", [NCORES * 2 * MEM, NL0], F32,
                                addr_space="Shared")

    with tile.TileContext(nc) as tc:
        with (
            tc.tile_pool(name="wpool", bufs=1) as wpool,
            tc.tile_pool(name="xpool", bufs=1) as xpool,
            tc.tile_pool(name="state", bufs=1) as state,
            tc.tile_pool(name="tmp", bufs=3) as tmp,
            tc.tile_pool(name="psA", bufs=3, space="PSUM") as psA,
            tc.tile_pool(name="psB", bufs=2, space="PSUM") as psB,
            tc.tile_pool(name="psF", bufs=2, space="PSUM") as psF,
        ):
            # ---- load everything ----
            wx_s = [wpool.tile([P, 4 * MEM], BF16, name=f"t", tag=f"wx{k}") for k in range(KT)]
            ws_s = [wpool.tile([P, 3 * MEM], BF16, name=f"t", tag=f"ws{k}") for k in range(KT)]
            wf_s = [wpool.tile([P, MEM], BF16, name=f"t", tag=f"wf{k}") for k in range(KT)]
            in_s = [wpool.tile([P, NSLOT], BF16, name=f"t", tag=f"in{k}") for k in range(KT)]
            for k in range(KT):
                r = slice(k * P, (k + 1) * P)
                nc.sync.dma_start(wx_s[k][:], wx_d[r, :])
                nc.sync.dma_start(in_s[k][:], xin_d[r, :])
                nc.sync.dma_start(ws_s[k][:], ws_d[r, :])
                nc.sync.dma_start(wf_s[k][:], wf_d[r, :])
            bx_s = wpool.tile([P, 16], F32, name="t", tag="bx")
            bs_s = wpool.tile([P, 12], F32, name="t", tag="bs")
            bf_s = wpool.tile([P, 4], F32, name="t", tag="bf")
            cm_s = wpool.tile([P, NL3], F32, name="t", tag="cm")
            nc.sync.dma_start(bx_s[:], bx_d[:])
            nc.sync.dma_start(bs_s[:], bs_d[:])
            nc.sync.dma_start(bf_s[:], bf_d[:])
            nc.sync.dma_start(cm_s[:], cm_d[:])

            # ---- phase A: X.T[2048, 704] = Wx.T @ xin (+bx) ----
            Xt = [xpool.tile([P, NSLOT], F32, name=f"t", tag=f"X{mc}") for mc in range(16)]
            for mc in range(16):
                for h in range(2):
                    ncols = slice(h * NHALF, (h + 1) * NHALF)
                    ps = psA.tile([P, NHALF], F32, name="t", tag="psA")
                    for k in range(KT):
                        nc.tensor.matmul(
                            ps[:],
                            wx_s[k][:, mc * P:(mc + 1) * P],
                            in_s[k][:, ncols],
                            start=(k == 0),
                            stop=(k == KT - 1),
                        )
                    # copy psum->sbuf with the bias add fused in (on DVE)
                    nc.vector.tensor_scalar_add(
                        Xt[mc][:, ncols], ps[:], bx_s[:, mc:mc + 1]
                    )

            # ---- rel3 leaf step ----
            H3 = [state.tile([P, NL3], BF16, name=f"t", tag=f"H3{m}") for m in range(KT)]
            C3 = [state.tile([P, NL3], F32, name=f"t", tag=f"C3{m}") for m in range(KT)]
            for m in range(KT):
                ig = tmp.tile([P, NL3], F32, name="t", tag="lf_i")
                og = tmp.tile([P, NL3], F32, name="t", tag="lf_o")
                ug = tmp.tile([P, NL3], F32, name="t", tag="lf_u")
                nc.scalar.activation(ig[:], Xt[m][:, OFF3:OFF3 + NL3],
                                     AF.Sigmoid, bias=bs_s[:, m:m + 1])
                nc.scalar.activation(og[:], Xt[8 + m][:, OFF3:OFF3 + NL3],
                                     AF.Sigmoid, bias=bs_s[:, 4 + m:5 + m])
                nc.scalar.activation(ug[:], Xt[12 + m][:, OFF3:OFF3 + NL3],
                                     AF.Tanh, bias=bs_s[:, 8 + m:9 + m])
                cr = tmp.tile([P, NL3], F32, name="t", tag="lf_c")
                nc.vector.tensor_mul(cr[:], ig[:], ug[:])
                nc.vector.tensor_mul(C3[m][:], cr[:], cm_s[:])  # zero pad slots
                th = tmp.tile([P, NL3], F32, name="t", tag="lf_t")
                nc.scalar.activation(th[:], C3[m][:], AF.Tanh)
                nc.vector.tensor_mul(H3[m][:], og[:], th[:])

            def level_step(n_par, x_off, Hc, Cc, hname, h_dtype=BF16):
                """One ChildSumTreeLSTM level: parents at X cols
                [x_off, x_off+n_par), children tiles Hc/Cc [128, 4*n_par]."""
                nch = 4 * n_par
                # f = sigmoid(Wf.T @ Hc + fx + bf); fccs = sum_children f*cc
                fccs = []
                for m in range(KT):
                    ps = psF.tile([P, nch], F32, name="t", tag="psF")
                    for k in range(KT):
                        nc.tensor.matmul(
                            ps[:], wf_s[k][:, m * P:(m + 1) * P], Hc[k][:],
                            start=(k == 0), stop=(k == KT - 1),
                        )
                    tf = tmp.tile([P, nch], F32, name="t", tag="st_tf")
                    pv = ps[:].rearrange("p (n g) -> p n g", g=4)
                    tv = tf[:].rearrange("p (n g) -> p n g", g=4)
                    fx = Xt[4 + m][:, x_off:x_off + n_par]
                    fxb = bass.AP(tensor=fx.tensor, offset=fx.offset,
                                  ap=list(fx.ap) + [[0, 4]])
                    nc.vector.tensor_add(tv[:], pv[:], fxb)
                    fg = tmp.tile([P, nch], F32, name="t", tag="st_fg")
                    nc.scalar.activation(fg[:], tf[:], AF.Sigmoid,
                                         bias=bf_s[:, m:m + 1])
                    fcc = tmp.tile([P, nch], F32, name="t", tag="st_fcc")
                    nc.vector.tensor_mul(fcc[:], fg[:], Cc[m][:])
                    fs = tmp.tile([P, n_par], F32, name="t", tag="st_fs")
                    nc.vector.tensor_reduce(
                        fs[:], fcc[:].rearrange("p (n g) -> p n g", g=4),
                        axis=AX.X, op=ALU.add,
                    )
                    fccs.append(fs)
                # child-h sum (groups of 4 adjacent columns)
                chs = [tmp.tile([P, n_par], BF16, name=f"t", tag=f"chs{k}") for k in range(KT)]
                for k in range(KT):
                    chf = tmp.tile([P, n_par], F32, name="t", tag="chf")
                    nc.vector.tensor_reduce(
                        chf[:],
                        Hc[k][:].rearrange("p (n g) -> p n g", g=4),
                        axis=AX.X, op=ALU.add,
                    )
                    nc.vector.tensor_copy(chs[k][:], chf[:])
                # iou.T = Ws.T @ chs (+bs)
                iou = [tmp.tile([P, n_par], F32, name=f"t", tag=f"iou{mc}") for mc in range(12)]
                for mc in range(12):
                    ps = psB.tile([P, n_par], F32, name="t", tag="psB")
                    for k in range(KT):
                        nc.tensor.matmul(
                            ps[:], ws_s[k][:, mc * P:(mc + 1) * P], chs[k][:],
                            start=(k == 0), stop=(k == KT - 1),
                        )
                    nc.vector.tensor_scalar_add(iou[mc][:], ps[:],
                                                bs_s[:, mc:mc + 1])
                Hp, Cp = [], []
                for m in range(KT):
                    pi = tmp.tile([P, n_par], F32, name="t", tag="st_pi")
                    po = tmp.tile([P, n_par], F32, name="t", tag="st_po")
                    pu = tmp.tile([P, n_par], F32, name="t", tag="st_pu")
                    nc.vector.tensor_add(pi[:], Xt[m][:, x_off:x_off + n_par],
                                         iou[m][:])
                    nc.vector.tensor_add(po[:], Xt[8 + m][:, x_off:x_off + n_par],
                                         iou[4 + m][:])
                    nc.vector.tensor_add(pu[:], Xt[12 + m][:, x_off:x_off + n_par],
                                         iou[8 + m][:])
                    ig = tmp.tile([P, n_par], F32, name="t", tag="st_ig")
                    og = tmp.tile([P, n_par], F32, name="t", tag="st_og")
                    ug = tmp.tile([P, n_par], F32, name="t", tag="st_ug")
                    nc.scalar.activation(ig[:], pi[:], AF.Sigmoid)
                    nc.scalar.activation(og[:], po[:], AF.Sigmoid)
                    nc.scalar.activation(ug[:], pu[:], AF.Tanh)
                    cp = state.tile([P, n_par], F32, name=f"t", tag=f"C_{hname}{m}")
                    iu = tmp.tile([P, n_par], F32, name="t", tag="st_iu")
                    nc.vector.tensor_mul(iu[:], ig[:], ug[:])
                    nc.vector.tensor_add(cp[:], iu[:], fccs[m][:])
                    th = tmp.tile([P, n_par], F32, name="t", tag="st_th")
                    nc.scalar.activation(th[:], cp[:], AF.Tanh)
                    hp = state.tile([P, n_par], h_dtype, name=f"t", tag=f"H_{hname}{m}")
                    nc.vector.tensor_mul(hp[:], og[:], th[:])
                    Hp.append(hp)
                    Cp.append(cp)
                return Hp, Cp

            H2, C2 = level_step(NL2, OFF2, H3, C3, "L2")
            H1, C1 = level_step(NL1, OFF1, H2, C2, "L1")
            H0, C0 = level_step(NL0, OFF0, H1, C1, "L0")

            if TOP_ON_DEVICE:
                # gather the 64 subtree roots (h and c) to every core
                for m in range(KT):
                    h0f = tmp.tile([P, NL0], F32, name="t", tag="h0f")
                    nc.vector.tensor_copy(h0f[:], H0[m][:])
                    nc.sync.dma_start(contrib_d[m * P:(m + 1) * P, :], h0f[:])
                    nc.sync.dma_start(contrib_d[MEM + m * P:MEM + (m + 1) * P, :],
                                      C0[m][:])
                nc.gpsimd.collective_compute(
                    "AllGather", ALU.bypass,
                    replica_groups=[list(range(NCORES))],
                    ins=[contrib_d[:]],
                    outs=[gath_d[:]],
                )
                H64f = [state.tile([P, 64], F32, name=f"t", tag=f"H64f{m}") for m in range(KT)]
                H64 = [state.tile([P, 64], BF16, name=f"t", tag=f"H64{m}") for m in range(KT)]
                C64 = [state.tile([P, 64], F32, name=f"t", tag=f"C64{m}") for m in range(KT)]
                # gath rows: 1024*r + 512*hc + 128*m + p ; cols: 8 roots
                gv = gath_d[:].rearrange("(r hc m p) c -> hc m p r c",
                                         r=NCORES, hc=2, m=KT)
                for m in range(KT):
                    nc.sync.dma_start(
                        H64f[m][:].rearrange("p (r c) -> p r c", r=NCORES),
                        gv[0, m],
                    )
                    nc.vector.tensor_copy(H64[m][:], H64f[m][:])
                    nc.sync.dma_start(
                        C64[m][:].rearrange("p (r c) -> p r c", r=NCORES),
                        gv[1, m],
                    )
                HT2, CT2 = level_step(16, OFFT2, H64, C64, "T2")
                HT1, CT1 = level_step(4, OFFT1, HT2, CT2, "T1")
                HT0, _ = level_step(1, OFFT0, HT1, CT1, "T0", h_dtype=F32)
                for m in range(KT):
                    nc.sync.dma_start(out_d[0, m * P:(m + 1) * P], HT0[m][:])
            else:
                for m in range(KT):
                    nc.sync.dma_start(roots_d[m * P:(m + 1) * P, :], H0[m][:])
                    nc.sync.dma_start(roots_d[MEM + m * P:MEM + (m + 1) * P, :],
                                      C0[m][:])
                z = wpool.tile([P, 4], F32, name="t", tag="zero")
                nc.vector.memset(z[:], 0.0)
                nc.sync.dma_start(out_d[0, :].rearrange("(m p) -> p m", p=P), z[:])

    nc.compile()
    return nc


_NC_CACHE = None


def kernel(inputs, Wx, bx, Ws, bs, Wf, bf, children):
    global LAST_RESULT, _NC_CACHE
    inputs = np.asarray(inputs, np.float32)
    Wx = np.asarray(Wx, np.float32)
    bx = np.asarray(bx, np.float32)
    Ws = np.asarray(Ws, np.float32)
    bs = np.asarray(bs, np.float32)
    Wf = np.asarray(Wf, np.float32)
    bf = np.asarray(bf, np.float32)

    Wx_b = Wx.astype(np.float16)
    Ws_b = Ws.astype(np.float16)
    Wf_b = Wf.astype(np.float16)
    bxT = np.ascontiguousarray(bx.reshape(16, P).T)
    bsT = np.ascontiguousarray(bs.reshape(12, P).T)
    bfT = np.ascontiguousarray(bf.reshape(4, P).T)

    in_maps = []
    core_masks = []
    for c in range(NCORES):
        heaps = _core_heaps(c)
        valid = (heaps >= 0) & (heaps < N)
        M = np.zeros((NSLOT, IN_DIM), np.float32)
        M[valid] = inputs[N - 1 - heaps[valid]]
        xin = np.ascontiguousarray(M.T)
        mrow = valid[:NL3].astype(np.float32)
        cmask = np.ascontiguousarray(np.tile(mrow[None, :], (P, 1)))
        core_masks.append(valid)
        in_maps.append({
            "xin": xin.astype(np.float16), "wx": Wx_b, "ws": Ws_b,
            "wf": Wf_b, "bx": bxT, "bs": bsT, "bf": bfT, "cmask": cmask,
        })

    if _NC_CACHE is None:
        _NC_CACHE = _build_program()
    nc = _NC_CACHE

    res = run_bass_kernel_spmd(
        nc, in_maps, list(range(NCORES)),
        trace=bool(os.environ.get("BASS_TRACE")),
    )
    LAST_RESULT = res

    if TOP_ON_DEVICE:
        return np.ascontiguousarray(res.results[0]["out"])

    # host fallback: finish the 21-node top tree in numpy
    Hr = np.zeros((64, MEM), np.float32)
    Cr = np.zeros((64, MEM), np.float32)
    for c in range(NCORES):
        r = res.results[c]["roots"]  # [1024, 8]
        Hr[8 * c:8 * c + 8] = r[:MEM].T
        Cr[8 * c:8 * c + 8] = r[MEM:].T

    def np_step(Hc, Cc, X_par):
        sig = lambda v: 1.0 / (1.0 + np.exp(-v))
        chs = Hc.reshape(-1, 4, MEM).sum(1)
        iou = chs @ Ws + bs
        i = sig(X_par[:, :MEM] + iou[:, :MEM])
        o = sig(X_par[:, 2 * MEM:3 * MEM] + iou[:, MEM:2 * MEM])
        u = np.tanh(X_par[:, 3 * MEM:] + iou[:, 2 * MEM:])
        fx = np.repeat(X_par[:, MEM:2 * MEM], 4, axis=0)
        f = sig(Hc @ Wf + bf + fx)
        cc = i * u + (f.reshape(-1, 4, MEM) * Cc.reshape(-1, 4, MEM)).sum(1)
        return o * np.tanh(cc), cc

    X_all = inputs @ Wx + bx  # [N, 2048] (only 21 rows used)
    Xtop = lambda hs: X_all[N - 1 - np.array(hs)]
    h2, c2 = np_step(Hr, Cr, Xtop(range(5, 21)))
    h1, c1 = np_step(h2, c2, Xtop(range(1, 5)))
    h0, _ = np_step(h1, c1, Xtop([0]))
    return np.ascontiguousarray(h0.astype(np.float32))



# revision 8
# speedup vs baseline: 1.0134x; 1.0134x over previous
"""ChildSumTreeLSTM on 8 trn2 NeuronCores — fused-level rewrite.

Tree: reversed complete 4-ary heap (id = N-1-heap; heap j's children 4j+1..4j+4).
Shard the 64 depth-3 subtrees across 8 cores (8 subtrees/core).  Per-core slot
array: leaf level 512 slots, then L2 128, L1 32, L0 8, plus the replicated top
tree T2 16 / T1 4 / T0 1 (+3 pad) = 704 slots.

Layout: mem dim (512 = 4 m-tiles of 128) on partitions, nodes on the free dim;
all state fused as single [128, 4m*n] tiles.  Gate order (i, u, o, f).
Gate preacts accumulate fully in PSUM (Wx/Ws GEMMs + K=1 bias-row matmul; a
per-core validity row zeroes pad leaf slots), so activations read PSUM
directly — no psum-evac + add chains on DVE.  X for level slots is computed
once into XLev and injected into level psums via identity matmuls.  The 64
subtree roots are AllGathered in fp16 node-major form (PE transposes) so the
reload is 2 contiguous DMAs, then every core computes the 21-node top tree.
"""

import os
import sys

sys.path.insert(0, "/opt/trn_rl_repo")

import numpy as np

import concourse.bass as bass
import concourse.bacc as bacc
import concourse.mybir as mybir
import concourse.tile as tile
from concourse.bass_utils import run_bass_kernel_spmd

F32 = mybir.dt.float32
F16 = mybir.dt.float16
AF = mybir.ActivationFunctionType
ALU = mybir.AluOpType
AX = mybir.AxisListType

N = 4096
MEM = 512
IN_DIM = 512
NCORES = 8
P = 128
KT = 4

# slot layout (gate order i,u,o,f throughout)
NL3 = 512
OFF3 = 0
NSLOT = 704
XLEV_BASE = 512          # XLev covers slots [512, 701)
XLEV_N = 189
# levels: (name, n_par, slot_off)
LEVELS = [
    ("L2", 128, 512),
    ("L1", 32, 640),
    ("L0", 8, 672),
    ("T2", 16, 680),
    ("T1", 4, 696),
    ("T0", 1, 700),
]
# XLevF (f-gate X replicated x4 per child) block offsets, widths 16*n
XF_OFF = {}
_off = 0
for _nm, _n, _ in LEVELS:
    XF_OFF[_nm] = _off
    _off += 16 * _n
XF_TOT = _off  # 3024

LAST_RESULT = None


def _core_heaps(c):
    t0 = 21 + 8 * c
    heaps = []
    for s in range(8):
        heaps += [64 * (t0 + s) + 21 + a for a in range(64)]  # leaf (depth 6)
    for s in range(8):
        heaps += [16 * (t0 + s) + 5 + a for a in range(16)]  # L2 (depth 5)
    for s in range(8):
        heaps += [4 * (t0 + s) + 1 + a for a in range(4)]  # L1 (depth 4)
    for s in range(8):
        heaps += [t0 + s]  # L0 (depth 3)
    heaps += list(range(5, 21)) + list(range(1, 5)) + [0]  # T2, T1, T0
    heaps += [-1, -1, -1]  # pad to 704
    return np.array(heaps, dtype=np.int64)


def _build_program():
    nc = bacc.Bacc("TRN2", target_bir_lowering=False, debug=False)

    xin_d = nc.dram_tensor("xin", [IN_DIM, NSLOT], F16, kind="ExternalInput")
    wx_d = nc.dram_tensor("wx", [IN_DIM, 4 * MEM], F16, kind="ExternalInput")
    ws_d = nc.dram_tensor("ws", [MEM, 3 * MEM], F16, kind="ExternalInput")
    wf_d = nc.dram_tensor("wf", [MEM, MEM], F16, kind="ExternalInput")
    brow_d = nc.dram_tensor("brow", [1, 4 * MEM], F16, kind="ExternalInput")
    vrow_d = nc.dram_tensor("vrow", [2, NL3], F16, kind="ExternalInput")
    id_d = nc.dram_tensor("ident", [P, P], F16, kind="ExternalInput")
    out_d = nc.dram_tensor("out", [1, MEM], F32, kind="ExternalOutput")
    contrib_d = nc.dram_tensor("contrib", [8, 2 * MEM], F16)
    gath_d = nc.dram_tensor("gath", [64, 2 * MEM], F16, addr_space="Shared")

    with tile.TileContext(nc) as tc:
        with (
            tc.tile_pool(name="wpool", bufs=1) as wpool,
            tc.tile_pool(name="spool", bufs=1) as spool,
            tc.tile_pool(name="psp", bufs=1, space="PSUM") as psp,
        ):
            # ---- psum: 3 rotating f32 tags (6 banks) + 1 f16 transpose tag ----
            ps_cnt = [0]

            def ps():
                t = psp.tile([P, 1024], F32, name="t", tag=f"ps{ps_cnt[0] % 3}")
                ps_cnt[0] += 1
                return t

            def ps16():
                return psp.tile([P, 1024], F16, name="t", tag="psT")

            # ---- loads (ordered for earliest leaf start) ----
            xin_s = [wpool.tile([P, NSLOT], F16, name="t", tag=f"in{k}") for k in range(KT)]
            wx_s = [wpool.tile([P, 4 * MEM], F16, name="t", tag=f"wx{k}") for k in range(KT)]
            ws_s = [wpool.tile([P, 3 * MEM], F16, name="t", tag=f"ws{k}") for k in range(KT)]
            wf_s = [wpool.tile([P, MEM], F16, name="t", tag=f"wf{k}") for k in range(KT)]
            brow_s = wpool.tile([1, 4 * MEM], F16, name="t", tag="brow")
            valid_s = wpool.tile([1, NL3], F16, name="t", tag="valid")
            ones_s = wpool.tile([1, NL3], F16, name="t", tag="ones")
            id_s = wpool.tile([P, P], F16, name="t", tag="id")

            for k in range(KT):
                r = slice(k * P, (k + 1) * P)
                nc.sync.dma_start(xin_s[k][:], xin_d[r, :])
            nc.sync.dma_start(brow_s[:], brow_d[:])
            nc.sync.dma_start(valid_s[:], vrow_d[0:1, :])
            nc.sync.dma_start(ones_s[:], vrow_d[1:2, :])
            nc.sync.dma_start(id_s[:], id_d[:])
            for k in range(KT):
                r = slice(k * P, (k + 1) * P)
                nc.sync.dma_start(wx_s[k][:, 0:1024], wx_d[r, 0:1024])  # i,u
            for k in range(KT):
                r = slice(k * P, (k + 1) * P)
                nc.sync.dma_start(wx_s[k][:, 1024:2048], wx_d[r, 1024:2048])  # o,f
            for k in range(KT):
                r = slice(k * P, (k + 1) * P)
                nc.sync.dma_start(wf_s[k][:], wf_d[r, :])
                nc.sync.dma_start(ws_s[k][:], ws_d[r, :])

            XLev = wpool.tile([P, 16 * XLEV_N], F16, name="t", tag="xlev")
            XLevF = wpool.tile([P, XF_TOT], F16, name="t", tag="xlevf")

            def bias_mm(pst, csl, mc, rtile, rsl, stop):
                """psum[:, csl] += brow[mc-tile].T @ rtile[0, rsl]  (K=1)."""
                nc.tensor.matmul(
                    pst[:, csl],
                    brow_s[0:1, mc * P:(mc + 1) * P],
                    rtile[0:1, rsl],
                    start=False, stop=stop,
                )

            # ================= leaf (2 halves of 256 slots) =================
            H3 = spool.tile([P, 4 * NL3], F16, name="t", tag="H3")
            C3 = spool.tile([P, 4 * NL3], F16, name="t", tag="C3")

            def leaf_gemm(pst, g, h):
                cols = slice(OFF3 + 256 * h, OFF3 + 256 * h + 256)
                for m in range(4):
                    osl = slice(m * 256, (m + 1) * 256)
                    mc = g * 4 + m
                    for k in range(KT):
                        nc.tensor.matmul(
                            pst[:, osl],
                            wx_s[k][:, mc * P:(mc + 1) * P],
                            xin_s[k][:, cols],
                            start=(k == 0), stop=False,
                        )
                    bias_mm(pst, osl, mc, valid_s, slice(256 * h, 256 * h + 256), True)

            Pi = [None, None]
            Pu = [None, None]
            for h in (0, 1):
                Pi[h] = ps()
                leaf_gemm(Pi[h], 0, h)
                Pu[h] = ps()
                leaf_gemm(Pu[h], 1, h)

            # XLev chunks i, u (after leaf i/u psums retire via ACTs below)
            Gi = [None, None]
            Gu = [None, None]
            Go = [None, None]
            th3 = [None, None]
            for h in (0, 1):
                Gi[h] = spool.tile([P, 1024], F16, name="t", tag=f"Gi{h}")
                Gu[h] = spool.tile([P, 1024], F16, name="t", tag=f"Gu{h}")
                nc.scalar.activation(Gi[h][:], Pi[h][:], AF.Sigmoid)
                nc.scalar.activation(Gu[h][:], Pu[h][:], AF.Tanh)
                c3v = bass.AP(
                    tensor=C3[:].tensor, offset=C3[:].offset + 256 * h,
                    ap=[list(C3[:].ap[0]), [NL3, 4], [1, 256]],
                )
                nc.vector.tensor_mul(
                    c3v, Gi[h][:].rearrange("p (m n) -> p m n", m=4),
                    Gu[h][:].rearrange("p (m n) -> p m n", m=4),
                )

            def xlev_gemm(g):
                pst = ps()
                for m in range(4):
                    osl = slice(m * XLEV_N, (m + 1) * XLEV_N)
                    mc = g * 4 + m
                    for k in range(KT):
                        nc.tensor.matmul(
                            pst[:, osl],
                            wx_s[k][:, mc * P:(mc + 1) * P],
                            xin_s[k][:, XLEV_BASE:XLEV_BASE + XLEV_N],
                            start=(k == 0), stop=False,
                        )
                    bias_mm(pst, osl, mc, ones_s, slice(0, XLEV_N), True)
                return pst

            PXi = xlev_gemm(0)
            PXu = xlev_gemm(1)
            nc.vector.tensor_copy(XLev[:, 0:756], PXi[:, 0:756])
            nc.vector.tensor_copy(XLev[:, 756:1512], PXu[:, 0:756])

            # leaf o gates + h
            Po = [None, None]
            for h in (0, 1):
                Po[h] = ps()
                leaf_gemm(Po[h], 2, h)
            for h in (0, 1):
                Go[h] = spool.tile([P, 1024], F16, name="t", tag=f"Go{h}")
                th3[h] = spool.tile([P, 1024], F16, name="t", tag=f"th3{h}")
                nc.scalar.activation(Go[h][:], Po[h][:], AF.Sigmoid)
                c3v = bass.AP(
                    tensor=C3[:].tensor, offset=C3[:].offset + 256 * h,
                    ap=[list(C3[:].ap[0]), [NL3, 4], [1, 256]],
                )
                nc.scalar.activation(
                    th3[h][:].rearrange("p (m n) -> p m n", m=4), c3v, AF.Tanh)
                h3v = bass.AP(
                    tensor=H3[:].tensor, offset=H3[:].offset + 256 * h,
                    ap=[list(H3[:].ap[0]), [NL3, 4], [1, 256]],
                )
                nc.vector.tensor_mul(
                    h3v, Go[h][:].rearrange("p (m n) -> p m n", m=4),
                    th3[h][:].rearrange("p (m n) -> p m n", m=4),
                )

            PXo = xlev_gemm(2)
            PXf = xlev_gemm(3)
            nc.scalar.copy(XLev[:, 1512:2268], PXo[:, 0:756])
            nc.vector.tensor_copy(XLev[:, 2268:3024], PXf[:, 0:756])

            # XLevF: replicate f-gate X x4 per child, per level
            for nm, n, soff in LEVELS:
                x_lo = soff - XLEV_BASE
                src = bass.AP(
                    tensor=XLev[:].tensor,
                    offset=XLev[:].offset + 12 * XLEV_N + x_lo,
                    ap=[list(XLev[:].ap[0]), [XLEV_N, 4], [1, n], [0, 4]],
                )
                dst = XLevF[:, XF_OFF[nm]:XF_OFF[nm] + 16 * n].rearrange(
                    "p (m n g) -> p m n g", m=4, g=4)
                if nm == "L2":
                    nc.gpsimd.tensor_copy(dst, src)
                elif nm in ("L0", "T1"):
                    nc.vector.tensor_copy(dst, src)
                else:
                    nc.scalar.copy(dst, src)

            # ================= fused level step =================
            def level_step(nm, n, soff, Hc, Cc, last=False):
                nch = 4 * n
                x_lo = soff - XLEV_BASE
                # child-h sum -> fp16
                chsf = spool.tile([P, nch], F32, name="t", tag=f"chsf{nm}")
                chs16 = spool.tile([P, nch], F16, name="t", tag=f"chs{nm}")
                nc.vector.tensor_reduce(
                    chsf[:].rearrange("p (m n) -> p m n", m=4),
                    Hc[:].rearrange("p (m n g) -> p m n g", m=4, g=4),
                    axis=AX.X, op=ALU.add,
                )
                nc.vector.tensor_copy(chs16[:], chsf[:])

                # forget path: psF = Wf.T @ Hc + Xf(rep) ; f = sigmoid(psF)
                f16 = spool.tile([P, 4 * nch], F16, name="t", tag=f"f{nm}")
                nparts = 2 if nch > 256 else 1
                mm_per = 4 // nparts
                for part in range(nparts):
                    pf = ps()
                    for mi in range(mm_per):
                        m = part * mm_per + mi
                        osl = slice(mi * nch, (mi + 1) * nch)
                        for k in range(KT):
                            nc.tensor.matmul(
                                pf[:, osl],
                                wf_s[k][:, m * P:(m + 1) * P],
                                Hc[:, k * nch:(k + 1) * nch],
                                start=(k == 0), stop=False,
                            )
                        nc.tensor.matmul(
                            pf[:, osl], id_s[:],
                            XLevF[:, XF_OFF[nm] + m * nch:XF_OFF[nm] + (m + 1) * nch],
                            start=False, stop=True,
                        )
                    nc.scalar.activation(
                        f16[:, part * mm_per * nch:(part + 1) * mm_per * nch],
                        pf[:, 0:mm_per * nch], AF.Sigmoid)

                # iou: psum (i,u,o) = XLev + Ws.T @ chs
                pio = ps()  # i cols 0:4n, o cols 4n:8n
                pu = ps()
                for g, pst, base in ((0, pio, 0), (1, pu, 0), (2, pio, nch)):
                    for m in range(4):
                        osl = slice(base + m * n, base + (m + 1) * n)
                        mc = g * 4 + m
                        nc.tensor.matmul(
                            pst[:, osl], id_s[:],
                            XLev[:, mc * XLEV_N + x_lo:mc * XLEV_N + x_lo + n],
                            start=True, stop=False,
                        )
                        for k in range(KT):
                            nc.tensor.matmul(
                                pst[:, osl],
                                ws_s[k][:, mc * P:(mc + 1) * P],
                                chs16[:, k * n:(k + 1) * n],
                                start=False, stop=(k == KT - 1),
                            )
                Gio = spool.tile([P, 2 * nch], F16, name="t", tag=f"Gio{nm}")
                Gu2 = spool.tile([P, nch], F16, name="t", tag=f"Gu2{nm}")
                nc.scalar.activation(Gio[:], pio[:, 0:2 * nch], AF.Sigmoid)
                nc.scalar.activation(Gu2[:], pu[:, 0:nch], AF.Tanh)

                # fcc = sum_children f * c
                fcc = spool.tile([P, 4 * nch], F16, name="t", tag=f"fcc{nm}")
                fs = spool.tile([P, nch], F32, name="t", tag=f"fs{nm}")
                nc.vector.tensor_mul(fcc[:], f16[:], Cc[:])
                nc.vector.tensor_reduce(
                    fs[:].rearrange("p (m n) -> p m n", m=4),
                    fcc[:].rearrange("p (m n g) -> p m n g", m=4, g=4),
                    axis=AX.X, op=ALU.add,
                )

                iu = spool.tile([P, nch], F16, name="t", tag=f"iu{nm}")
                Cp = spool.tile([P, nch], F16, name="t", tag=f"C{nm}")
                thp = spool.tile([P, nch], F16, name="t", tag=f"th{nm}")
                Hp = spool.tile([P, nch], F16, name="t", tag=f"H{nm}")
                nc.vector.tensor_mul(iu[:], Gio[:, 0:nch], Gu2[:])
                nc.vector.tensor_add(Cp[:], iu[:], fs[:])
                nc.scalar.activation(thp[:], Cp[:], AF.Tanh)
                nc.vector.tensor_mul(Hp[:], Gio[:, nch:2 * nch], thp[:])
                return Hp, Cp

            H2, C2 = level_step("L2", 128, 512, H3, C3)
            H1, C1 = level_step("L1", 32, 640, H2, C2)
            H0, C0 = level_step("L0", 8, 672, H1, C1)

            # ============ collective: roots to node-major, AllGather ============
            PT = ps16()  # [8, 1024]: h mem 0:512, c mem 512:1024
            for m in range(KT):
                nc.tensor.transpose(
                    PT[0:8, m * P:(m + 1) * P], H0[:, m * 8:(m + 1) * 8], id_s[:])
                nc.tensor.transpose(
                    PT[0:8, MEM + m * P:MEM + (m + 1) * P], C0[:, m * 8:(m + 1) * 8],
                    id_s[:])
            contribT = spool.tile([8, 2 * MEM], F16, name="t", tag="contribT")
            nc.scalar.copy(contribT[:], PT[0:8, :])
            nc.sync.dma_start(contrib_d[:], contribT[:])
            nc.gpsimd.collective_compute(
                "AllGather", ALU.bypass,
                replica_groups=[list(range(NCORES))],
                ins=[contrib_d[:]],
                outs=[gath_d[:]],
            )
            Hnm = spool.tile([64, MEM], F16, name="t", tag="Hnm")
            Cnm = spool.tile([64, MEM], F16, name="t", tag="Cnm")
            nc.sync.dma_start(Hnm[:], gath_d[:, 0:MEM])
            nc.sync.dma_start(Cnm[:], gath_d[:, MEM:2 * MEM])
            PT2 = ps16()  # [128, 512]: H64 cols 0:256, C64 cols 256:512
            for m in range(KT):
                nc.tensor.transpose(
                    PT2[:, m * 64:(m + 1) * 64], Hnm[:, m * P:(m + 1) * P],
                    id_s[0:64, 0:64])
                nc.tensor.transpose(
                    PT2[:, 256 + m * 64:256 + (m + 1) * 64], Cnm[:, m * P:(m + 1) * P],
                    id_s[0:64, 0:64])
            H64 = spool.tile([P, 256], F16, name="t", tag="H64")
            C64 = spool.tile([P, 256], F16, name="t", tag="C64")
            nc.vector.tensor_copy(H64[:], PT2[:, 0:256])
            nc.vector.tensor_copy(C64[:], PT2[:, 256:512])

            # ================= top tree =================
            HT2, CT2 = level_step("T2", 16, 680, H64, C64)
            HT1, CT1 = level_step("T1", 4, 696, HT2, CT2)
            HT0, _ = level_step("T0", 1, 700, HT1, CT1, last=True)

            PT3 = ps16()
            nc.tensor.transpose(PT3[0:4, 0:P], HT0[:, 0:4], id_s[:])
            out_sb = spool.tile([4, P], F32, name="t", tag="outsb")
            nc.scalar.copy(out_sb[:], PT3[0:4, 0:P])
            nc.sync.dma_start(
                out_d[0, :].rearrange("(a b) -> a b", a=4), out_sb[:])

    nc.compile()
    return nc


_NC_CACHE = None


def kernel(inputs, Wx, bx, Ws, bs, Wf, bf, children):
    global LAST_RESULT, _NC_CACHE
    inputs = np.asarray(inputs, np.float32)
    Wx = np.asarray(Wx, np.float32)
    bx = np.asarray(bx, np.float32)
    Ws = np.asarray(Ws, np.float32)
    bs = np.asarray(bs, np.float32)
    Wf = np.asarray(Wf, np.float32)
    bf = np.asarray(bf, np.float32)

    M2 = MEM
    # gate order (i, u, o, f); natural Wx order (i, f, o, u), Ws (i, o, u)
    Wxp = np.concatenate(
        [Wx[:, 0:M2], Wx[:, 3 * M2:4 * M2], Wx[:, 2 * M2:3 * M2], Wx[:, M2:2 * M2]], 1)
    Wsp = np.concatenate([Ws[:, 0:M2], Ws[:, 2 * M2:3 * M2], Ws[:, M2:2 * M2]], 1)
    bxp = np.concatenate([bx[0:M2], bx[3 * M2:4 * M2], bx[2 * M2:3 * M2], bx[M2:2 * M2]])
    bsp = np.concatenate([bs[0:M2], bs[2 * M2:3 * M2], bs[M2:2 * M2]])
    brow = np.zeros(4 * M2, np.float32)
    brow[0:3 * M2] = bxp[0:3 * M2] + bsp
    brow[3 * M2:] = bxp[3 * M2:] + bf
    ident = np.eye(P, dtype=np.float16)

    Wx16 = Wxp.astype(np.float16)
    Ws16 = Wsp.astype(np.float16)
    Wf16 = Wf.astype(np.float16)
    brow16 = brow[None, :].astype(np.float16)

    in_maps = []
    for c in range(NCORES):
        heaps = _core_heaps(c)
        valid = (heaps >= 0) & (heaps < N)
        M = np.zeros((NSLOT, IN_DIM), np.float32)
        M[valid] = inputs[N - 1 - heaps[valid]]
        xin = np.ascontiguousarray(M.T).astype(np.float16)
        vrow = np.zeros((2, NL3), np.float16)
        vrow[0] = valid[:NL3].astype(np.float16)
        vrow[1] = 1.0
        in_maps.append({
            "xin": xin, "wx": Wx16, "ws": Ws16, "wf": Wf16,
            "brow": brow16, "vrow": vrow, "ident": ident,
        })

    if _NC_CACHE is None:
        _NC_CACHE = _build_program()
    nc = _NC_CACHE

    res = run_bass_kernel_spmd(
        nc, in_maps, list(range(NCORES)),
        trace=bool(os.environ.get("BASS_TRACE")),
    )
    LAST_RESULT = res
    return np.ascontiguousarray(res.results[0]["out"])


# revision 14
# speedup vs baseline: 1.2397x; 1.2232x over previous
"""ChildSumTreeLSTM on 8 trn2 NeuronCores — fused-level rewrite, v2.

Tree: reversed complete 4-ary heap (id = N-1-heap; heap j's children 4j+1..4j+4).
The 64 depth-3 subtrees are assigned round-robin (core c gets heaps 21+8s+c,
s=0..7) so real leaf work balances: only subtrees with heap<=63 have depth-6
leaves, so every core has at most 6 leafy subtrees -> leaf level = 384 slots.
Per-core slot array: leaf 384, L2 128, L1 32, L0 8, T2 16, T1 4, T0 1, pad 3
= 576 slots.

Layout: mem dim (512 = 4 m-tiles of 128) on partitions, nodes on free dim; all
state fused as single [128, 4m*n] tiles, gate order (i, u, o, f).  One merged
Wx GEMM computes leaf preacts and XLev (X of all level slots) per (k, mc) pair;
leaf gates activate straight from PSUM (per-m bias via ACT bias column).  Level
steps add XLev into the Ws-GEMM psum with one DVE tensor_tensor (stride-0
broadcast for the f-gate), so the PE runs only real GEMM pairs.  The 64 subtree
roots are AllGathered in f32 node-major form (PE transposes); the strided
assignment is undone by 8 strided reload DMAs per tensor.  Every core then
computes the 21-node top tree.
"""

import os
import sys

sys.path.insert(0, "/opt/trn_rl_repo")

import numpy as np

import concourse.bass as bass
import concourse.bacc as bacc
import concourse.mybir as mybir
import concourse.tile as tile
from concourse.bass_utils import run_bass_kernel_spmd

F32 = mybir.dt.float32
F16 = mybir.dt.float16
AF = mybir.ActivationFunctionType
ALU = mybir.AluOpType
AX = mybir.AxisListType

N = 4096
MEM = 512
IN_DIM = 512
NCORES = 8
P = 128
KT = 4

NL3 = 384                # leaf slots (6 leafy subtrees x 64)
NSLOT = 576
XLEV_BASE = 384          # XLev covers slots [384, 573)
XLEV_N = 189
W573 = NL3 + XLEV_N      # merged leaf+XLev GEMM width per mc

LAST_RESULT = None


def _core_heaps(c):
    # subtree s of core c is rooted at heap 21 + 8*s + c (round-robin)
    t = [21 + 8 * s + c for s in range(8)]
    heaps = []
    for s in range(6):
        heaps += [64 * t[s] + 21 + a for a in range(64)]  # leaf (depth 6)
    for s in range(8):
        heaps += [16 * t[s] + 5 + a for a in range(16)]  # L2 (depth 5)
    for s in range(8):
        heaps += [4 * t[s] + 1 + a for a in range(4)]  # L1 (depth 4)
    for s in range(8):
        heaps += [t[s]]  # L0 (depth 3)
    heaps += list(range(5, 21)) + list(range(1, 5)) + [0]  # T2, T1, T0
    heaps += [-1, -1, -1]  # pad to 576
    return np.array(heaps, dtype=np.int64)


def _build_program():
    nc = bacc.Bacc("TRN2", target_bir_lowering=False, debug=False)

    xin_d = nc.dram_tensor("xin", [IN_DIM, NSLOT], F16, kind="ExternalInput")
    wx_d = nc.dram_tensor("wx", [IN_DIM, 4 * MEM], F16, kind="ExternalInput")
    ws_d = nc.dram_tensor("ws", [MEM, 3 * MEM], F16, kind="ExternalInput")
    wf_d = nc.dram_tensor("wf", [MEM, MEM], F16, kind="ExternalInput")
    bcol_d = nc.dram_tensor("bcol", [P, 16], F32, kind="ExternalInput")
    cm_d = nc.dram_tensor("cmask", [P, NL3], F16, kind="ExternalInput")
    id_d = nc.dram_tensor("ident", [P, P], F32, kind="ExternalInput")
    out_d = nc.dram_tensor("out", [1, MEM], F32, kind="ExternalOutput")
    DBG = bool(os.environ.get("KERNEL_DEBUG"))
    if DBG:
        xlev_dbg = nc.dram_tensor("xlev_dbg", [P, 16 * XLEV_N], F16, kind="ExternalOutput")
        h3_dbg = nc.dram_tensor("h3_dbg", [P, 2048], F16, kind="ExternalOutput")
        c3_dbg = nc.dram_tensor("c3_dbg", [P, 2048], F16, kind="ExternalOutput")
        h2_dbg = nc.dram_tensor("h2_dbg", [P, 512], F16, kind="ExternalOutput")
        f2_dbg = nc.dram_tensor("f2_dbg", [P, 2048], F16, kind="ExternalOutput")
        pre2_dbg = nc.dram_tensor("pre2_dbg", [P, 1536], F16, kind="ExternalOutput")
        h0_dbg = nc.dram_tensor("h0_dbg", [P, 32], F32, kind="ExternalOutput")
        hnm_dbg = nc.dram_tensor("hnm_dbg", [64, MEM], F32, kind="ExternalOutput")
        h64_dbg = nc.dram_tensor("h64_dbg", [P, 256], F16, kind="ExternalOutput")
        ht2_dbg = nc.dram_tensor("ht2_dbg", [P, 64], F16, kind="ExternalOutput")
    contrib_d = nc.dram_tensor("contrib", [8, 2 * MEM], F32)
    gath_d = nc.dram_tensor("gath", [64, 2 * MEM], F32, addr_space="Shared")

    with tile.TileContext(nc) as tc:
        with (
            tc.tile_pool(name="wpool", bufs=1) as wpool,
            tc.tile_pool(name="spool", bufs=1) as spool,
            tc.tile_pool(name="psp", bufs=1, space="PSUM") as psp,
        ):
            ps_cnt = [0]

            def ps():
                t = psp.tile([P, 2048], F32, name="t", tag=f"ps{ps_cnt[0] % 2}")
                ps_cnt[0] += 1
                return t

            # ---- loads (ordered for earliest start) ----
            xin_s = [wpool.tile([P, NSLOT], F16, name="t", tag=f"in{k}") for k in range(KT)]
            wx_s = [wpool.tile([P, 4 * MEM], F16, name="t", tag=f"wx{k}") for k in range(KT)]
            ws_s = [wpool.tile([P, 3 * MEM], F16, name="t", tag=f"ws{k}") for k in range(KT)]
            wf_s = [wpool.tile([P, MEM], F16, name="t", tag=f"wf{k}") for k in range(KT)]
            bcol_s = wpool.tile([P, 16], F32, name="t", tag="bcol")
            cm_s = wpool.tile([P, NL3], F16, name="t", tag="cm")
            id_s = wpool.tile([P, P], F32, name="t", tag="id")

            for k in range(KT):
                r = slice(k * P, (k + 1) * P)
                nc.sync.dma_start(xin_s[k][:], xin_d[r, :])
            nc.sync.dma_start(bcol_s[:], bcol_d[:])
            nc.sync.dma_start(cm_s[:], cm_d[:])
            nc.sync.dma_start(id_s[:], id_d[:])
            for k in range(KT):
                r = slice(k * P, (k + 1) * P)
                nc.sync.dma_start(wx_s[k][:, 0:1024], wx_d[r, 0:1024])  # i,u
            for k in range(KT):
                r = slice(k * P, (k + 1) * P)
                nc.sync.dma_start(wx_s[k][:, 1024:2048], wx_d[r, 1024:2048])  # o,f
            for k in range(KT):
                r = slice(k * P, (k + 1) * P)
                nc.sync.dma_start(wf_s[k][:], wf_d[r, :])
                nc.sync.dma_start(ws_s[k][:], ws_d[r, :])

            XLev = wpool.tile([P, 16 * XLEV_N], F16, name="t", tag="xlev")

            # ========== leaf + XLev: one merged Wx GEMM ==========
            H3 = spool.tile([P, 4 * 512], F16, name="t", tag="H3")
            C3 = spool.tile([P, 4 * 512], F16, name="t", tag="C3")
            nc.vector.memset(H3[:], 0.0)
            nc.vector.memset(C3[:], 0.0)

            # gates i, u, o: psum [128, 2*573] per (gate, m-pair)
            Gt = {}
            for g in range(3):
                Gt[g] = spool.tile([P, 4 * NL3], F16, name="t", tag=f"G{g}")
            for g, fn in ((0, AF.Sigmoid), (1, AF.Tanh), (2, AF.Sigmoid)):
                for mp in (0, 1):  # m pairs (0,1) and (2,3)
                    pst = ps()
                    for mi in (0, 1):
                        m = 2 * mp + mi
                        mc = g * 4 + m
                        # matmul psum output must not cross a 1024-f32 (4KB)
                        # boundary: leaf chunk at mi*512, XLev chunk in the
                        # upper half at 1024 + 192*mi
                        for k in range(KT):
                            nc.tensor.matmul(
                                pst[:, mi * 512:mi * 512 + NL3],
                                wx_s[k][:, mc * P:(mc + 1) * P],
                                xin_s[k][:, 0:NL3],
                                start=(k == 0), stop=(k == KT - 1),
                            )
                            nc.tensor.matmul(
                                pst[:, 1024 + 192 * mi:1024 + 192 * mi + XLEV_N],
                                wx_s[k][:, mc * P:(mc + 1) * P],
                                xin_s[k][:, NL3:W573],
                                start=(k == 0), stop=(k == KT - 1),
                            )
                    for mi in (0, 1):
                        m = 2 * mp + mi
                        mc = g * 4 + m
                        # leaf part: activate straight from psum (+bias)
                        nc.scalar.activation(
                            Gt[g][:, m * NL3:(m + 1) * NL3],
                            pst[:, mi * 512:mi * 512 + NL3],
                            fn, bias=bcol_s[:, mc:mc + 1])
                        # XLev part: psum + bias -> fp16 (split scalar/vector)
                        xsl = slice(mc * XLEV_N, (mc + 1) * XLEV_N)
                        psl = pst[:, 1024 + 192 * mi:1024 + 192 * mi + XLEV_N]
                        if mi == 0:
                            nc.vector.tensor_scalar_add(
                                XLev[:, xsl], psl, bcol_s[:, mc:mc + 1])
                        else:
                            nc.scalar.add(XLev[:, xsl], psl, bcol_s[:, mc:mc + 1])
            # f gate: XLev only
            pst = ps()
            for m in range(4):
                mc = 12 + m
                # one 189-wide region per 512-f32 psum bank (no bank crossing)
                osl = slice(m * 512, m * 512 + XLEV_N)
                for k in range(KT):
                    nc.tensor.matmul(
                        pst[:, osl],
                        wx_s[k][:, mc * P:(mc + 1) * P],
                        xin_s[k][:, XLEV_BASE:XLEV_BASE + XLEV_N],
                        start=(k == 0), stop=(k == KT - 1),
                    )
                xsl = slice(mc * XLEV_N, (mc + 1) * XLEV_N)
                if m % 2 == 0:
                    nc.vector.tensor_scalar_add(
                        XLev[:, xsl], pst[:, osl], bcol_s[:, mc:mc + 1])
                else:
                    nc.scalar.add(XLev[:, xsl], pst[:, osl], bcol_s[:, mc:mc + 1])

            # leaf elementwise: c = (i*u)*mask, h = o*tanh(c)
            c3v = bass.AP(tensor=C3[:].tensor, offset=C3[:].offset,
                          ap=[list(C3[:].ap[0]), [512, 4], [1, NL3]])
            h3v = bass.AP(tensor=H3[:].tensor, offset=H3[:].offset,
                          ap=[list(H3[:].ap[0]), [512, 4], [1, NL3]])
            cmv = bass.AP(tensor=cm_s[:].tensor, offset=cm_s[:].offset,
                          ap=[list(cm_s[:].ap[0]), [0, 4], [1, NL3]])
            iu3 = spool.tile([P, 4 * NL3], F16, name="t", tag="iu3")
            th3 = spool.tile([P, 4 * NL3], F16, name="t", tag="th3")
            nc.vector.tensor_mul(iu3[:], Gt[0][:], Gt[1][:])
            nc.vector.tensor_mul(
                c3v, iu3[:].rearrange("p (m n) -> p m n", m=4), cmv)
            nc.scalar.activation(
                th3[:].rearrange("p (m n) -> p m n", m=4), c3v, AF.Tanh)
            nc.vector.tensor_mul(h3v,
                                 Gt[2][:].rearrange("p (m n) -> p m n", m=4),
                                 th3[:].rearrange("p (m n) -> p m n", m=4))

            _lvdbg = {}

            # ========== fused level step ==========
            def level_step(nm, n, soff, Hc, Cc, nch_stride, h_dtype=F16):
                """Hc/Cc: [128, 4m*nch_stride] tiles; children in cols [0, 4n)."""
                nch = 4 * n
                x_lo = soff - XLEV_BASE

                def cview(t, inner):
                    return bass.AP(
                        tensor=t[:].tensor, offset=t[:].offset,
                        ap=[list(t[:].ap[0]), [nch_stride, 4]] + inner)

                # child-h sum -> fp16
                chsf = spool.tile([P, nch], F32, name="t", tag=f"chsf{nm}")
                chs16 = spool.tile([P, nch], F16, name="t", tag=f"chs{nm}")
                nc.vector.tensor_reduce(
                    chsf[:].rearrange("p (m n) -> p m n", m=4),
                    cview(Hc, [[4, n], [1, 4]]),
                    axis=AX.X, op=ALU.add,
                )
                nc.vector.tensor_copy(chs16[:], chsf[:])

                # forget path: psF = Wf.T @ Hc ; pre_f = psF + XLevF ; sigmoid
                pf = ps()
                for m in range(4):
                    osl = slice(m * nch, (m + 1) * nch)
                    for k in range(KT):
                        nc.tensor.matmul(
                            pf[:, osl],
                            wf_s[k][:, m * P:(m + 1) * P],
                            Hc[:, k * nch_stride:k * nch_stride + nch],
                            start=(k == 0), stop=(k == KT - 1),
                        )
                f16 = spool.tile([P, 4 * nch], F16, name="t", tag=f"f{nm}")
                xfv = bass.AP(
                    tensor=XLev[:].tensor,
                    offset=XLev[:].offset + 12 * XLEV_N + x_lo,
                    ap=[list(XLev[:].ap[0]), [XLEV_N, 4], [1, n], [0, 4]])
                nc.vector.tensor_add(
                    f16[:].rearrange("p (m n g) -> p m n g", m=4, g=4),
                    pf[:, 0:4 * nch].rearrange("p (m n g) -> p m n g", m=4, g=4),
                    xfv)
                nc.scalar.activation(f16[:], f16[:], AF.Sigmoid)

                # iou: psum = Ws.T @ chs ; pre = psum + XLev ; activate
                piou = ps()
                for g in range(3):
                    for m in range(4):
                        osl = slice((g * 4 + m) * n, (g * 4 + m + 1) * n)
                        mc = g * 4 + m
                        for k in range(KT):
                            nc.tensor.matmul(
                                piou[:, osl],
                                ws_s[k][:, mc * P:(mc + 1) * P],
                                chs16[:, k * n:(k + 1) * n],
                                start=(k == 0), stop=(k == KT - 1),
                            )
                pre = spool.tile([P, 12 * n], F16, name="t", tag=f"pre{nm}")
                xv = bass.AP(
                    tensor=XLev[:].tensor, offset=XLev[:].offset + x_lo,
                    ap=[list(XLev[:].ap[0]), [XLEV_N, 12], [1, n]])
                nc.vector.tensor_add(
                    pre[:].rearrange("p (c n) -> p c n", c=12),
                    piou[:, 0:12 * n].rearrange("p (c n) -> p c n", c=12), xv)
                Gio = spool.tile([P, 2 * nch], F16, name="t", tag=f"Gio{nm}")
                Gu2 = spool.tile([P, nch], F16, name="t", tag=f"Gu2{nm}")
                iov = bass.AP(
                    tensor=pre[:].tensor, offset=pre[:].offset,
                    ap=[list(pre[:].ap[0]), [8 * n, 2], [1, nch]])
                nc.scalar.activation(
                    Gio[:].rearrange("p (a b) -> p a b", a=2), iov, AF.Sigmoid)
                nc.scalar.activation(Gu2[:], pre[:, nch:2 * nch], AF.Tanh)

                # fcc = sum_children f * c
                fcc = spool.tile([P, 4 * nch], F16, name="t", tag=f"fcc{nm}")
                fs = spool.tile([P, nch], F32, name="t", tag=f"fs{nm}")
                nc.vector.tensor_mul(
                    fcc[:].rearrange("p (m c) -> p m c", m=4),
                    f16[:].rearrange("p (m c) -> p m c", m=4),
                    cview(Cc, [[1, nch]]))
                nc.vector.tensor_reduce(
                    fs[:].rearrange("p (m n) -> p m n", m=4),
                    fcc[:].rearrange("p (m n g) -> p m n g", m=4, g=4),
                    axis=AX.X, op=ALU.add,
                )

                iu = spool.tile([P, nch], F16, name="t", tag=f"iu{nm}")
                Cp = spool.tile([P, nch], F16, name="t", tag=f"C{nm}")
                thp = spool.tile([P, nch], F16, name="t", tag=f"th{nm}")
                Hp = spool.tile([P, nch], h_dtype, name="t", tag=f"H{nm}")
                nc.vector.tensor_mul(iu[:], Gio[:, 0:nch], Gu2[:])
                nc.vector.tensor_add(Cp[:], iu[:], fs[:])
                nc.scalar.activation(thp[:], Cp[:], AF.Tanh)
                nc.vector.tensor_mul(Hp[:], Gio[:, nch:2 * nch], thp[:])
                _lvdbg[nm] = (f16, pre)
                return Hp, Cp

            H2, C2 = level_step("L2", 128, 384, H3, C3, 512)
            H1, C1 = level_step("L1", 32, 512, H2, C2, 128)
            H0f, C0f = level_step("L0", 8, 544, H1, C1, 32)
            # f32 copies of the 8 roots for the f32 transpose/collective
            H0 = spool.tile([P, 32], F32, name="t", tag="H0c")
            C0 = spool.tile([P, 32], F32, name="t", tag="C0c")
            nc.vector.tensor_copy(H0[:], H0f[:])
            nc.vector.tensor_copy(C0[:], C0f[:])

            # ===== collective: roots -> node-major f32, AllGather =====
            PT = ps()  # [8, 1024]: h mem 0:512, c mem 512:1024
            for m in range(KT):
                nc.tensor.transpose(
                    PT[0:8, m * P:(m + 1) * P], H0[:, m * 8:(m + 1) * 8], id_s[:])
                nc.tensor.transpose(
                    PT[0:8, MEM + m * P:MEM + (m + 1) * P], C0[:, m * 8:(m + 1) * 8],
                    id_s[:])
            contribT = spool.tile([8, 2 * MEM], F32, name="t", tag="contribT")
            nc.scalar.copy(contribT[:], PT[0:8, 0:2 * MEM])
            nc.sync.dma_start(contrib_d[:], contribT[:])
            nc.gpsimd.collective_compute(
                "AllGather", ALU.bypass,
                replica_groups=[list(range(NCORES))],
                ins=[contrib_d[:]],
                outs=[gath_d[:]],
            )
            # reload in heap order: root heap 21+g, g=8s+c sits at gather row 8c+s
            Hnm = spool.tile([64, MEM], F32, name="t", tag="Hnm")
            Cnm = spool.tile([64, MEM], F32, name="t", tag="Cnm")
            gv = gath_d[:].rearrange("(c s) f -> s c f", c=8)
            for s in range(8):
                nc.sync.dma_start(Hnm[8 * s:8 * s + 8, :], gv[s, :, 0:MEM])
                nc.sync.dma_start(Cnm[8 * s:8 * s + 8, :], gv[s, :, MEM:2 * MEM])
            PT2 = ps()  # [128, 512]: H64 cols 0:256, C64 cols 256:512
            for m in range(KT):
                nc.tensor.transpose(
                    PT2[:, m * 64:(m + 1) * 64], Hnm[:, m * P:(m + 1) * P],
                    id_s[0:64, 0:64])
                nc.tensor.transpose(
                    PT2[:, 256 + m * 64:256 + (m + 1) * 64], Cnm[:, m * P:(m + 1) * P],
                    id_s[0:64, 0:64])
            H64 = spool.tile([P, 256], F16, name="t", tag="H64")
            C64 = spool.tile([P, 256], F16, name="t", tag="C64")
            nc.vector.tensor_copy(H64[:], PT2[:, 0:256])
            nc.vector.tensor_copy(C64[:], PT2[:, 256:512])

            # ================= top tree =================
            HT2, CT2 = level_step("T2", 16, 552, H64, C64, 64)
            HT1, CT1 = level_step("T1", 4, 568, HT2, CT2, 16)
            HT0, _ = level_step("T0", 1, 572, HT1, CT1, 4, h_dtype=F32)

            if DBG:
                nc.sync.dma_start(xlev_dbg[:], XLev[:])
                nc.sync.dma_start(h3_dbg[:], H3[:])
                nc.sync.dma_start(c3_dbg[:], C3[:])
                nc.sync.dma_start(h2_dbg[:], H2[:])
                nc.sync.dma_start(f2_dbg[:], _lvdbg["L2"][0][:])
                nc.sync.dma_start(pre2_dbg[:], _lvdbg["L2"][1][:])
                nc.sync.dma_start(h0_dbg[:], H0[:])
                nc.sync.dma_start(hnm_dbg[:], Hnm[:])
                nc.sync.dma_start(h64_dbg[:], H64[:])
                nc.sync.dma_start(ht2_dbg[:], HT2[:])

            PT3 = ps()
            nc.tensor.transpose(PT3[0:4, 0:P], HT0[:, 0:4], id_s[:])
            out_sb = spool.tile([4, P], F32, name="t", tag="outsb")
            nc.scalar.copy(out_sb[:], PT3[0:4, 0:P])
            nc.sync.dma_start(
                out_d[0, :].rearrange("(a b) -> a b", a=4), out_sb[:])

    nc.compile()
    return nc


_NC_CACHE = None


def kernel(inputs, Wx, bx, Ws, bs, Wf, bf, children):
    global LAST_RESULT, _NC_CACHE
    inputs = np.asarray(inputs, np.float32)
    Wx = np.asarray(Wx, np.float32)
    bx = np.asarray(bx, np.float32)
    Ws = np.asarray(Ws, np.float32)
    bs = np.asarray(bs, np.float32)
    Wf = np.asarray(Wf, np.float32)
    bf = np.asarray(bf, np.float32)

    M2 = MEM
    # gate order (i, u, o, f); natural Wx order (i, f, o, u), Ws (i, o, u)
    Wxp = np.concatenate(
        [Wx[:, 0:M2], Wx[:, 3 * M2:4 * M2], Wx[:, 2 * M2:3 * M2], Wx[:, M2:2 * M2]], 1)
    Wsp = np.concatenate([Ws[:, 0:M2], Ws[:, 2 * M2:3 * M2], Ws[:, M2:2 * M2]], 1)
    bxp = np.concatenate([bx[0:M2], bx[3 * M2:4 * M2], bx[2 * M2:3 * M2], bx[M2:2 * M2]])
    bsp = np.concatenate([bs[0:M2], bs[2 * M2:3 * M2], bs[M2:2 * M2]])
    brow = np.zeros(4 * M2, np.float32)
    brow[0:3 * M2] = bxp[0:3 * M2] + bsp
    brow[3 * M2:] = bxp[3 * M2:] + bf
    bcol = np.ascontiguousarray(brow.reshape(16, P).T)

    Wx16 = Wxp.astype(np.float16)
    Ws16 = Wsp.astype(np.float16)
    Wf16 = Wf.astype(np.float16)
    ident = np.eye(P, dtype=np.float32)

    in_maps = []
    for c in range(NCORES):
        heaps = _core_heaps(c)
        valid = (heaps >= 0) & (heaps < N)
        M = np.zeros((NSLOT, IN_DIM), np.float32)
        M[valid] = inputs[N - 1 - heaps[valid]]
        xin = np.ascontiguousarray(M.T).astype(np.float16)
        cmask = np.ascontiguousarray(
            np.tile(valid[:NL3].astype(np.float16)[None, :], (P, 1)))
        in_maps.append({
            "xin": xin, "wx": Wx16, "ws": Ws16, "wf": Wf16,
            "bcol": bcol, "cmask": cmask, "ident": ident,
        })

    if _NC_CACHE is None:
        _NC_CACHE = _build_program()
    nc = _NC_CACHE

    res = run_bass_kernel_spmd(
        nc, in_maps, list(range(NCORES)),
        trace=bool(os.environ.get("BASS_TRACE")),
    )
    LAST_RESULT = res
    return np.ascontiguousarray(res.results[0]["out"])


# revision 16
# speedup vs baseline: 1.3505x; 1.0895x over previous
"""ChildSumTreeLSTM on 8 trn2 NeuronCores — fused-level rewrite, v2.

Tree: reversed complete 4-ary heap (id = N-1-heap; heap j's children 4j+1..4j+4).
The 64 depth-3 subtrees are assigned round-robin (core c gets heaps 21+8s+c,
s=0..7) so real leaf work balances: only subtrees with heap<=63 have depth-6
leaves, so every core has at most 6 leafy subtrees -> leaf level = 384 slots.
Per-core slot array: leaf 384, L2 128, L1 32, L0 8, T2 16, T1 4, T0 1, pad 3
= 576 slots.

Layout: mem dim (512 = 4 m-tiles of 128) on partitions, nodes on free dim; all
state fused as single [128, 4m*n] tiles, gate order (i, u, o, f).  One merged
Wx GEMM computes leaf preacts and XLev (X of all level slots) per (k, mc) pair;
leaf gates activate straight from PSUM (per-m bias via ACT bias column).  Level
steps add XLev into the Ws-GEMM psum with one DVE tensor_tensor (stride-0
broadcast for the f-gate), so the PE runs only real GEMM pairs.  The 64 subtree
roots are AllGathered in f32 node-major form (PE transposes); the strided
assignment is undone by 8 strided reload DMAs per tensor.  Every core then
computes the 21-node top tree.
"""

import os
import sys

sys.path.insert(0, "/opt/trn_rl_repo")

import numpy as np

import concourse.bass as bass
import concourse.bacc as bacc
import concourse.mybir as mybir
import concourse.tile as tile
from concourse.bass_utils import run_bass_kernel_spmd

F32 = mybir.dt.float32
F16 = mybir.dt.float16
AF = mybir.ActivationFunctionType
ALU = mybir.AluOpType
AX = mybir.AxisListType

N = 4096
MEM = 512
IN_DIM = 512
NCORES = 8
P = 128
KT = 4

NL3 = 384                # leaf slots (6 leafy subtrees x 64)
NSLOT = 576
XLEV_BASE = 384          # XLev covers slots [384, 573)
XLEV_N = 189
W573 = NL3 + XLEV_N      # merged leaf+XLev GEMM width per mc

LAST_RESULT = None


def _core_heaps(c):
    # subtree s of core c is rooted at heap 21 + 8*s + c (round-robin)
    t = [21 + 8 * s + c for s in range(8)]
    heaps = []
    for s in range(6):
        heaps += [64 * t[s] + 21 + a for a in range(64)]  # leaf (depth 6)
    for s in range(8):
        heaps += [16 * t[s] + 5 + a for a in range(16)]  # L2 (depth 5)
    for s in range(8):
        heaps += [4 * t[s] + 1 + a for a in range(4)]  # L1 (depth 4)
    for s in range(8):
        heaps += [t[s]]  # L0 (depth 3)
    heaps += list(range(5, 21)) + list(range(1, 5)) + [0]  # T2, T1, T0
    heaps += [-1, -1, -1]  # pad to 576
    return np.array(heaps, dtype=np.int64)


def _build_program():
    nc = bacc.Bacc("TRN2", target_bir_lowering=False, debug=False)

    xin_d = nc.dram_tensor("xin", [P, KT * NSLOT], F16, kind="ExternalInput")
    wxiu_d = nc.dram_tensor("wxiu", [P, KT * 1024], F16, kind="ExternalInput")
    wxof_d = nc.dram_tensor("wxof", [P, KT * 1024], F16, kind="ExternalInput")
    ws_d = nc.dram_tensor("ws", [P, KT * 3 * MEM], F16, kind="ExternalInput")
    wf_d = nc.dram_tensor("wf", [P, KT * MEM], F16, kind="ExternalInput")
    bcol_d = nc.dram_tensor("bcol", [P, 16], F32, kind="ExternalInput")
    cm_d = nc.dram_tensor("cmask", [P, NL3], F16, kind="ExternalInput")
    id_d = nc.dram_tensor("ident", [P, P], F32, kind="ExternalInput")
    out_d = nc.dram_tensor("out", [1, MEM], F32, kind="ExternalOutput")
    DBG = bool(os.environ.get("KERNEL_DEBUG"))
    if DBG:
        xlev_dbg = nc.dram_tensor("xlev_dbg", [P, 16 * XLEV_N], F16, kind="ExternalOutput")
        h3_dbg = nc.dram_tensor("h3_dbg", [P, 2048], F16, kind="ExternalOutput")
        c3_dbg = nc.dram_tensor("c3_dbg", [P, 2048], F16, kind="ExternalOutput")
        h2_dbg = nc.dram_tensor("h2_dbg", [P, 512], F16, kind="ExternalOutput")
        f2_dbg = nc.dram_tensor("f2_dbg", [P, 2048], F16, kind="ExternalOutput")
        pre2_dbg = nc.dram_tensor("pre2_dbg", [P, 1536], F16, kind="ExternalOutput")
        h0_dbg = nc.dram_tensor("h0_dbg", [P, 32], F32, kind="ExternalOutput")
        hnm_dbg = nc.dram_tensor("hnm_dbg", [64, MEM], F32, kind="ExternalOutput")
        h64_dbg = nc.dram_tensor("h64_dbg", [P, 256], F16, kind="ExternalOutput")
        ht2_dbg = nc.dram_tensor("ht2_dbg", [P, 64], F16, kind="ExternalOutput")
    contrib_d = nc.dram_tensor("contrib", [8, 2 * MEM], F32)
    gath_d = nc.dram_tensor("gath", [64, 2 * MEM], F32, addr_space="Shared")

    with tile.TileContext(nc) as tc:
        with (
            tc.tile_pool(name="wpool", bufs=1) as wpool,
            tc.tile_pool(name="spool", bufs=1) as spool,
            tc.tile_pool(name="psp", bufs=1, space="PSUM") as psp,
        ):
            ps_cnt = [0]

            def ps():
                t = psp.tile([P, 2048], F32, name="t", tag=f"ps{ps_cnt[0] % 2}")
                ps_cnt[0] += 1
                return t

            # ---- loads: one big k-major DMA per tensor ----
            xin_t = wpool.tile([P, KT * NSLOT], F16, name="t", tag="xin")
            wxiu_t = wpool.tile([P, KT * 1024], F16, name="t", tag="wxiu")
            wxof_t = wpool.tile([P, KT * 1024], F16, name="t", tag="wxof")
            ws_t = wpool.tile([P, KT * 3 * MEM], F16, name="t", tag="ws")
            wf_t = wpool.tile([P, KT * MEM], F16, name="t", tag="wf")
            bcol_s = wpool.tile([P, 16], F32, name="t", tag="bcol")
            cm_s = wpool.tile([P, NL3], F16, name="t", tag="cm")
            id_s = wpool.tile([P, P], F32, name="t", tag="id")

            nc.sync.dma_start(xin_t[:], xin_d[:])
            nc.sync.dma_start(wxiu_t[:], wxiu_d[:])
            nc.sync.dma_start(bcol_s[:], bcol_d[:])
            nc.sync.dma_start(cm_s[:], cm_d[:])
            nc.sync.dma_start(id_s[:], id_d[:])
            nc.sync.dma_start(wxof_t[:], wxof_d[:])
            nc.sync.dma_start(wf_t[:], wf_d[:])
            nc.sync.dma_start(ws_t[:], ws_d[:])

            def wx_sl(k, mc, width=P):
                # wx lhsT tile for (k, mc): gates i,u in wxiu, o,f in wxof
                if mc < 8:
                    return wxiu_t[:, k * 1024 + mc * P:k * 1024 + mc * P + width]
                return wxof_t[:, k * 1024 + (mc - 8) * P:k * 1024 + (mc - 8) * P + width]

            XLev = wpool.tile([P, 16 * XLEV_N], F16, name="t", tag="xlev")

            # ========== leaf + XLev: one merged Wx GEMM ==========
            H3 = spool.tile([P, 4 * 512], F16, name="t", tag="H3")
            C3 = spool.tile([P, 4 * 512], F16, name="t", tag="C3")
            nc.vector.memset(H3[:], 0.0)
            nc.vector.memset(C3[:], 0.0)

            # gates i, u, o: psum [128, 2*573] per (gate, m-pair)
            Gt = {}
            for g in range(3):
                Gt[g] = spool.tile([P, 4 * NL3], F16, name="t", tag=f"G{g}")
            for g, fn in ((0, AF.Sigmoid), (1, AF.Tanh), (2, AF.Sigmoid)):
                for mp in (0, 1):  # m pairs (0,1) and (2,3)
                    pst = ps()
                    for mi in (0, 1):
                        m = 2 * mp + mi
                        mc = g * 4 + m
                        # matmul psum output must not cross a 1024-f32 (4KB)
                        # boundary: leaf chunk at mi*512, XLev chunk in the
                        # upper half at 1024 + 192*mi
                        for k in range(KT):
                            nc.tensor.matmul(
                                pst[:, mi * 512:mi * 512 + NL3],
                                wx_sl(k, mc),
                                xin_t[:, k * NSLOT:k * NSLOT + NL3],
                                start=(k == 0), stop=(k == KT - 1),
                            )
                            nc.tensor.matmul(
                                pst[:, 1024 + 192 * mi:1024 + 192 * mi + XLEV_N],
                                wx_sl(k, mc),
                                xin_t[:, k * NSLOT + NL3:k * NSLOT + W573],
                                start=(k == 0), stop=(k == KT - 1),
                            )
                    for mi in (0, 1):
                        m = 2 * mp + mi
                        mc = g * 4 + m
                        # leaf part: activate straight from psum (+bias)
                        nc.scalar.activation(
                            Gt[g][:, m * NL3:(m + 1) * NL3],
                            pst[:, mi * 512:mi * 512 + NL3],
                            fn, bias=bcol_s[:, mc:mc + 1])
                        # XLev part: psum + bias -> fp16 (split scalar/vector)
                        xsl = slice(mc * XLEV_N, (mc + 1) * XLEV_N)
                        psl = pst[:, 1024 + 192 * mi:1024 + 192 * mi + XLEV_N]
                        if mi == 0:
                            nc.vector.tensor_scalar_add(
                                XLev[:, xsl], psl, bcol_s[:, mc:mc + 1])
                        else:
                            nc.scalar.add(XLev[:, xsl], psl, bcol_s[:, mc:mc + 1])
            # f gate: XLev only
            pst = ps()
            for m in range(4):
                mc = 12 + m
                # one 189-wide region per 512-f32 psum bank (no bank crossing)
                osl = slice(m * 512, m * 512 + XLEV_N)
                for k in range(KT):
                    nc.tensor.matmul(
                        pst[:, osl],
                        wx_sl(k, mc),
                        xin_t[:, k * NSLOT + XLEV_BASE:k * NSLOT + XLEV_BASE + XLEV_N],
                        start=(k == 0), stop=(k == KT - 1),
                    )
                xsl = slice(mc * XLEV_N, (mc + 1) * XLEV_N)
                if m % 2 == 0:
                    nc.vector.tensor_scalar_add(
                        XLev[:, xsl], pst[:, osl], bcol_s[:, mc:mc + 1])
                else:
                    nc.scalar.add(XLev[:, xsl], pst[:, osl], bcol_s[:, mc:mc + 1])

            # leaf elementwise: c = (i*u)*mask, h = o*tanh(c)
            c3v = bass.AP(tensor=C3[:].tensor, offset=C3[:].offset,
                          ap=[list(C3[:].ap[0]), [512, 4], [1, NL3]])
            h3v = bass.AP(tensor=H3[:].tensor, offset=H3[:].offset,
                          ap=[list(H3[:].ap[0]), [512, 4], [1, NL3]])
            cmv = bass.AP(tensor=cm_s[:].tensor, offset=cm_s[:].offset,
                          ap=[list(cm_s[:].ap[0]), [0, 4], [1, NL3]])
            iu3 = spool.tile([P, 4 * NL3], F16, name="t", tag="iu3")
            th3 = spool.tile([P, 4 * NL3], F16, name="t", tag="th3")
            nc.vector.tensor_mul(iu3[:], Gt[0][:], Gt[1][:])
            nc.vector.tensor_mul(
                c3v, iu3[:].rearrange("p (m n) -> p m n", m=4), cmv)
            nc.scalar.activation(
                th3[:].rearrange("p (m n) -> p m n", m=4), c3v, AF.Tanh)
            nc.vector.tensor_mul(h3v,
                                 Gt[2][:].rearrange("p (m n) -> p m n", m=4),
                                 th3[:].rearrange("p (m n) -> p m n", m=4))

            _lvdbg = {}

            # ========== fused level step ==========
            def level_step(nm, n, soff, Hc, Cc, nch_stride, h_dtype=F16):
                """Hc/Cc: [128, 4m*nch_stride] tiles; children in cols [0, 4n)."""
                nch = 4 * n
                x_lo = soff - XLEV_BASE

                def cview(t, inner):
                    return bass.AP(
                        tensor=t[:].tensor, offset=t[:].offset,
                        ap=[list(t[:].ap[0]), [nch_stride, 4]] + inner)

                # child-h sum -> fp16
                chsf = spool.tile([P, nch], F32, name="t", tag=f"chsf{nm}")
                chs16 = spool.tile([P, nch], F16, name="t", tag=f"chs{nm}")
                nc.vector.tensor_reduce(
                    chsf[:].rearrange("p (m n) -> p m n", m=4),
                    cview(Hc, [[4, n], [1, 4]]),
                    axis=AX.X, op=ALU.add,
                )
                nc.vector.tensor_copy(chs16[:], chsf[:])

                # forget path: psF = Wf.T @ Hc ; pre_f = psF + XLevF ; sigmoid
                pf = ps()
                for m in range(4):
                    osl = slice(m * nch, (m + 1) * nch)
                    for k in range(KT):
                        nc.tensor.matmul(
                            pf[:, osl],
                            wf_t[:, k * MEM + m * P:k * MEM + (m + 1) * P],
                            Hc[:, k * nch_stride:k * nch_stride + nch],
                            start=(k == 0), stop=(k == KT - 1),
                        )
                f16 = spool.tile([P, 4 * nch], F16, name="t", tag=f"f{nm}")
                xfv = bass.AP(
                    tensor=XLev[:].tensor,
                    offset=XLev[:].offset + 12 * XLEV_N + x_lo,
                    ap=[list(XLev[:].ap[0]), [XLEV_N, 4], [1, n], [0, 4]])
                nc.vector.tensor_add(
                    f16[:].rearrange("p (m n g) -> p m n g", m=4, g=4),
                    pf[:, 0:4 * nch].rearrange("p (m n g) -> p m n g", m=4, g=4),
                    xfv)
                nc.scalar.activation(f16[:], f16[:], AF.Sigmoid)

                # iou: psum = Ws.T @ chs ; pre = psum + XLev ; activate
                piou = ps()
                for g in range(3):
                    for m in range(4):
                        osl = slice((g * 4 + m) * n, (g * 4 + m + 1) * n)
                        mc = g * 4 + m
                        for k in range(KT):
                            nc.tensor.matmul(
                                piou[:, osl],
                                ws_t[:, k * 1536 + mc * P:k * 1536 + (mc + 1) * P],
                                chs16[:, k * n:(k + 1) * n],
                                start=(k == 0), stop=(k == KT - 1),
                            )
                pre = spool.tile([P, 12 * n], F16, name="t", tag=f"pre{nm}")
                xv = bass.AP(
                    tensor=XLev[:].tensor, offset=XLev[:].offset + x_lo,
                    ap=[list(XLev[:].ap[0]), [XLEV_N, 12], [1, n]])
                nc.vector.tensor_add(
                    pre[:].rearrange("p (c n) -> p c n", c=12),
                    piou[:, 0:12 * n].rearrange("p (c n) -> p c n", c=12), xv)
                Gio = spool.tile([P, 2 * nch], F16, name="t", tag=f"Gio{nm}")
                Gu2 = spool.tile([P, nch], F16, name="t", tag=f"Gu2{nm}")
                iov = bass.AP(
                    tensor=pre[:].tensor, offset=pre[:].offset,
                    ap=[list(pre[:].ap[0]), [8 * n, 2], [1, nch]])
                nc.scalar.activation(
                    Gio[:].rearrange("p (a b) -> p a b", a=2), iov, AF.Sigmoid)
                nc.scalar.activation(Gu2[:], pre[:, nch:2 * nch], AF.Tanh)

                # fcc = sum_children f * c
                fcc = spool.tile([P, 4 * nch], F16, name="t", tag=f"fcc{nm}")
                fs = spool.tile([P, nch], F32, name="t", tag=f"fs{nm}")
                nc.vector.tensor_mul(
                    fcc[:].rearrange("p (m c) -> p m c", m=4),
                    f16[:].rearrange("p (m c) -> p m c", m=4),
                    cview(Cc, [[1, nch]]))
                nc.vector.tensor_reduce(
                    fs[:].rearrange("p (m n) -> p m n", m=4),
                    fcc[:].rearrange("p (m n g) -> p m n g", m=4, g=4),
                    axis=AX.X, op=ALU.add,
                )

                iu = spool.tile([P, nch], F16, name="t", tag=f"iu{nm}")
                Cp = spool.tile([P, nch], F16, name="t", tag=f"C{nm}")
                thp = spool.tile([P, nch], F16, name="t", tag=f"th{nm}")
                Hp = spool.tile([P, nch], h_dtype, name="t", tag=f"H{nm}")
                nc.vector.tensor_mul(iu[:], Gio[:, 0:nch], Gu2[:])
                nc.vector.tensor_add(Cp[:], iu[:], fs[:])
                nc.scalar.activation(thp[:], Cp[:], AF.Tanh)
                nc.vector.tensor_mul(Hp[:], Gio[:, nch:2 * nch], thp[:])
                _lvdbg[nm] = (f16, pre)
                return Hp, Cp

            H2, C2 = level_step("L2", 128, 384, H3, C3, 512)
            H1, C1 = level_step("L1", 32, 512, H2, C2, 128)
            H0f, C0f = level_step("L0", 8, 544, H1, C1, 32)
            # f32 copies of the 8 roots for the f32 transpose/collective
            H0 = spool.tile([P, 32], F32, name="t", tag="H0c")
            C0 = spool.tile([P, 32], F32, name="t", tag="C0c")
            nc.vector.tensor_copy(H0[:], H0f[:])
            nc.vector.tensor_copy(C0[:], C0f[:])

            # ===== collective: roots -> node-major f32, AllGather =====
            PT = ps()  # [8, 1024]: h mem 0:512, c mem 512:1024
            for m in range(KT):
                nc.tensor.transpose(
                    PT[0:8, m * P:(m + 1) * P], H0[:, m * 8:(m + 1) * 8], id_s[:])
                nc.tensor.transpose(
                    PT[0:8, MEM + m * P:MEM + (m + 1) * P], C0[:, m * 8:(m + 1) * 8],
                    id_s[:])
            contribT = spool.tile([8, 2 * MEM], F32, name="t", tag="contribT")
            nc.scalar.copy(contribT[:], PT[0:8, 0:2 * MEM])
            nc.sync.dma_start(contrib_d[:], contribT[:])
            nc.gpsimd.collective_compute(
                "AllGather", ALU.bypass,
                replica_groups=[list(range(NCORES))],
                ins=[contrib_d[:]],
                outs=[gath_d[:]],
            )
            # reload in gather order (rows 8c+s); heap order restored in the
            # psum->sbuf copies below via permuted APs
            Hnm = spool.tile([64, MEM], F32, name="t", tag="Hnm")
            Cnm = spool.tile([64, MEM], F32, name="t", tag="Cnm")
            nc.sync.dma_start(Hnm[:], gath_d[:, 0:MEM])
            nc.sync.dma_start(Cnm[:], gath_d[:, MEM:2 * MEM])
            # keep the PE's activity monitor warm during the collective so
            # the top-tree matmuls run at full clock
            warm = ps()
            for w in range(28):
                nc.tensor.matmul(
                    warm[:, 0:512],
                    wf_t[:, 0:P],
                    xin_t[:, 0:512],
                    start=True, stop=True,
                )

            PT2 = ps()  # [128, 512]: H64 cols 0:256, C64 cols 256:512
            for m in range(KT):
                nc.tensor.transpose(
                    PT2[:, m * 64:(m + 1) * 64], Hnm[:, m * P:(m + 1) * P],
                    id_s[0:64, 0:64])
                nc.tensor.transpose(
                    PT2[:, 256 + m * 64:256 + (m + 1) * 64], Cnm[:, m * P:(m + 1) * P],
                    id_s[0:64, 0:64])
            H64 = spool.tile([P, 256], F16, name="t", tag="H64")
            C64 = spool.tile([P, 256], F16, name="t", tag="C64")
            # psum col m*64 + 8c+s  ->  sbuf col m*64 + 8s+c  (heap order)
            for base, dst in ((0, H64), (256, C64)):
                srcv = bass.AP(
                    tensor=PT2[:].tensor, offset=PT2[:].offset + base,
                    ap=[list(PT2[:].ap[0]), [64, 4], [1, 8], [8, 8]])
                dstv = dst[:].rearrange("p (m s c) -> p m s c", m=4, s=8)
                nc.vector.tensor_copy(dstv, srcv)

            # ================= top tree =================
            HT2, CT2 = level_step("T2", 16, 552, H64, C64, 64)
            HT1, CT1 = level_step("T1", 4, 568, HT2, CT2, 16)
            HT0, _ = level_step("T0", 1, 572, HT1, CT1, 4, h_dtype=F32)

            if DBG:
                nc.sync.dma_start(xlev_dbg[:], XLev[:])
                nc.sync.dma_start(h3_dbg[:], H3[:])
                nc.sync.dma_start(c3_dbg[:], C3[:])
                nc.sync.dma_start(h2_dbg[:], H2[:])
                nc.sync.dma_start(f2_dbg[:], _lvdbg["L2"][0][:])
                nc.sync.dma_start(pre2_dbg[:], _lvdbg["L2"][1][:])
                nc.sync.dma_start(h0_dbg[:], H0[:])
                nc.sync.dma_start(hnm_dbg[:], Hnm[:])
                nc.sync.dma_start(h64_dbg[:], H64[:])
                nc.sync.dma_start(ht2_dbg[:], HT2[:])

            PT3 = ps()
            nc.tensor.transpose(PT3[0:4, 0:P], HT0[:, 0:4], id_s[:])
            out_sb = spool.tile([4, P], F32, name="t", tag="outsb")
            nc.scalar.copy(out_sb[:], PT3[0:4, 0:P])
            nc.sync.dma_start(
                out_d[0, :].rearrange("(a b) -> a b", a=4), out_sb[:])

    nc.compile()
    return nc


_NC_CACHE = None


def kernel(inputs, Wx, bx, Ws, bs, Wf, bf, children):
    global LAST_RESULT, _NC_CACHE
    inputs = np.asarray(inputs, np.float32)
    Wx = np.asarray(Wx, np.float32)
    bx = np.asarray(bx, np.float32)
    Ws = np.asarray(Ws, np.float32)
    bs = np.asarray(bs, np.float32)
    Wf = np.asarray(Wf, np.float32)
    bf = np.asarray(bf, np.float32)

    M2 = MEM
    # gate order (i, u, o, f); natural Wx order (i, f, o, u), Ws (i, o, u)
    Wxp = np.concatenate(
        [Wx[:, 0:M2], Wx[:, 3 * M2:4 * M2], Wx[:, 2 * M2:3 * M2], Wx[:, M2:2 * M2]], 1)
    Wsp = np.concatenate([Ws[:, 0:M2], Ws[:, 2 * M2:3 * M2], Ws[:, M2:2 * M2]], 1)
    bxp = np.concatenate([bx[0:M2], bx[3 * M2:4 * M2], bx[2 * M2:3 * M2], bx[M2:2 * M2]])
    bsp = np.concatenate([bs[0:M2], bs[2 * M2:3 * M2], bs[M2:2 * M2]])
    brow = np.zeros(4 * M2, np.float32)
    brow[0:3 * M2] = bxp[0:3 * M2] + bsp
    brow[3 * M2:] = bxp[3 * M2:] + bf
    bcol = np.ascontiguousarray(brow.reshape(16, P).T)

    def kmaj(a):
        # [512, F] -> [128, 4*F]  (k-major: col k*F + f)
        Fd = a.shape[1]
        return np.ascontiguousarray(
            a.reshape(KT, P, Fd).transpose(1, 0, 2).reshape(P, KT * Fd))

    Wx16 = Wxp.astype(np.float16)
    wxiu = kmaj(Wx16[:, 0:1024])
    wxof = kmaj(Wx16[:, 1024:2048])
    ws16 = kmaj(Wsp.astype(np.float16))
    wf16 = kmaj(Wf.astype(np.float16))
    ident = np.eye(P, dtype=np.float32)

    in_maps = []
    for c in range(NCORES):
        heaps = _core_heaps(c)
        valid = (heaps >= 0) & (heaps < N)
        M = np.zeros((NSLOT, IN_DIM), np.float32)
        M[valid] = inputs[N - 1 - heaps[valid]]
        xin = kmaj(np.ascontiguousarray(M.T).astype(np.float16))
        cmask = np.ascontiguousarray(
            np.tile(valid[:NL3].astype(np.float16)[None, :], (P, 1)))
        in_maps.append({
            "xin": xin, "wxiu": wxiu, "wxof": wxof, "ws": ws16, "wf": wf16,
            "bcol": bcol, "cmask": cmask, "ident": ident,
        })

    if _NC_CACHE is None:
        _NC_CACHE = _build_program()
    nc = _NC_CACHE

    res = run_bass_kernel_spmd(
        nc, in_maps, list(range(NCORES)),
        trace=bool(os.environ.get("BASS_TRACE")),
    )
    LAST_RESULT = res
    return np.ascontiguousarray(res.results[0]["out"])
